# revision 27
# baseline (speedup 1.0000x reference)
"""AUGRU (attention-update GRU) Trainium2 kernel.

Problem: T=200, B=1024, D=128 AUGRU scan; final state [B, D] output.

Strategy:
  - Data-parallel over batch: 8 cores x 128 batch each (SPMD, same program).
  - Per-core layout is TRANSPOSED: [D(partitions)=128, B(free)=128].
    All matmuls are out = W.T @ xT (lhsT = W as stored), so the recurrent
    state never needs a transpose on-chip.
  - Per step t, one PSUM bank holds [zu | zr | xc | sc] (4 x 128 cols):
      zu = xu + s@Wbu   (PSUM accumulation: proj matmul start=True, then
      zr = xr + s@Wbr    state matmul start=False accumulates for free)
      xc, sc kept separate (r gates sc before xc is added).
  - sigmoid([zu|zr]) is ONE activation op over 256 cols.
  - ma[t,b] = mask[b,t]*att[t,b,0] is precomputed on host; broadcast to
    128 partitions on-chip via a K=1 matmul (ones[1,128].T @ ma_row[1,B]),
    batched 4 steps per bank.
  - Final combine: s' = s + ma*u*(c-s)  (equivalent to the reference's
    masked convex-combination update).
"""

import numpy as np
from contextlib import ExitStack

T, B, D = 200, 1024, 128
NCORES = 8
BS = B // NCORES          # 128 batch per core
CH = 20                   # time steps per x DMA chunk
NCH = T // CH             # 10 chunks
MA_GROUP = 4              # steps of ma broadcast per K=1 matmul

_PROGRAM_CACHE = {}


def _build_program(use_bias: bool):
    import concourse.bass as bass
    import concourse.bacc as bacc
    import concourse.tile as tile
    from concourse import mybir
    from concourse.tile import add_dep_helper

    f32 = mybir.dt.float32
    AF = mybir.ActivationFunctionType

    nc = bacc.Bacc("TRN2", target_bir_lowering=False)

    x_d = nc.declare_dram_parameter("x", [NCH, D, CH * BS], f32, isOutput=False)
    s0_d = nc.declare_dram_parameter("s0", [D, BS], f32, isOutput=False)
    ma_d = nc.declare_dram_parameter("ma", [1, T * BS + D], f32, isOutput=False)
    z_d = nc.declare_dram_parameter("zconst", [D, D], f32, isOutput=False)
    w_names = ["wau", "war", "wac", "wbu", "wbr", "wbc"]
    w_d = {n: nc.declare_dram_parameter(n, [D, D], f32, isOutput=False) for n in w_names}
    if use_bias:
        b_names = ["bau", "bar", "bac"]
        b_d = {n: nc.declare_dram_parameter(n, [D, 1], f32, isOutput=False) for n in b_names}
    out_d = nc.declare_dram_parameter("sout", [D, BS], f32, isOutput=True)

    with ExitStack() as ctx:
        tc = ctx.enter_context(tile.TileContext(nc))
        consts = ctx.enter_context(tc.tile_pool(name="consts", bufs=1))
        xpool = ctx.enter_context(tc.tile_pool(name="xpool", bufs=2))
        spool = ctx.enter_context(tc.tile_pool(name="spool", bufs=3))
        ew = ctx.enter_context(tc.tile_pool(name="ew", bufs=3))
        apsum = ctx.enter_context(tc.tile_pool(name="apsum", bufs=4, space="PSUM"))
        bpsum = ctx.enter_context(tc.tile_pool(name="bpsum", bufs=3, space="PSUM"))
        scpsum = ctx.enter_context(tc.tile_pool(name="scpsum", bufs=1, space="PSUM"))
        mabc_pool = ctx.enter_context(tc.tile_pool(name="mabc_pool", bufs=1))

        wt = {}
        for n in w_names:
            wt[n] = consts.tile([D, D], f32, name=f"w_{n}", tag=f"w_{n}")
            nc.sync.dma_start(out=wt[n], in_=w_d[n][:, :])
        bt = {}
        if use_bias:
            for n in b_names:
                bt[n] = consts.tile([D, 1], f32, name=f"b_{n}", tag=f"b_{n}")
                nc.sync.dma_start(out=bt[n], in_=b_d[n][:, :])
        zeros = consts.tile([D, D], f32, name="zeros", tag="zeros")
        nc.sync.dma_start(out=zeros, in_=z_d[:, :])
        # Pre-broadcast all of ma to 128 partitions into persistent SBUF
        # tiles (partition-stride-0 SWDGE DMAs). Never recycled => readers
        # carry at most the one DMA wait on first use.
        mabc_all = []
        for g in range(NCH):
            mt = mabc_pool.tile([D, CH * BS], f32, name=f"mabc{g}", tag=f"mabc{g}")
            srcap = ma_d[:, g * CH * BS:(g + 1) * CH * BS]
            bcast = bass.AP(tensor=srcap.tensor, offset=srcap.offset,
                            ap=[[0, D]] + list(srcap.ap[1:]))
            nc.gpsimd.dma_start(out=mt, in_=bcast)
            mabc_all.append(mt)

        s = spool.tile([D, BS], f32, name="s", tag="s")
        nc.sync.dma_start(out=s, in_=s0_d[:, :])
        scratch = scpsum.tile([D, 8], f32, name="scratch", tag="scratch")
        prev = nc.tensor.matmul(scratch[:, 0:2], lhsT=zeros, rhs=zeros[:, 0:2],
                                start=True, stop=True)
        for n in w_names:
            d = nc.tensor.matmul(scratch[:, 0:2], lhsT=wt[n], rhs=zeros[:, 0:2],
                                 start=True, stop=True)
            add_dep_helper(d.ins, prev.ins, sync=False, reason="startup dma absorb chain")
            prev = d
        d = nc.tensor.matmul(scratch[:, 0:2], lhsT=zeros, rhs=s[:, 0:2],
                             start=True, stop=True)
        add_dep_helper(d.ins, prev.ins, sync=False, reason="startup dma absorb chain")
        startup_absorber = d

        pma = None
        for ich in range(NCH):
            xch = xpool.tile([D, CH * BS], f32, name="xch", tag="xch")
            nc.sync.dma_start(out=xch, in_=x_d[ich])
            for j in range(CH):
                t = ich * CH + j
                x_t = xch[:, j * BS:(j + 1) * BS]

                if j == 0:
                    # Chunk head: a zero-valued matmul into a PE-only
                    # scratch bank absorbs the x-chunk DMA wait so real
                    # matmuls carry at most one cross-engine sync wait.
                    mmz = nc.tensor.matmul(
                        scratch[:, 0:2], lhsT=zeros, rhs=xch[:, 0:2],
                        start=True, stop=True,
                    )
                    if ich == 0:
                        add_dep_helper(mmz.ins, startup_absorber.ins, sync=False,
                                       reason="after startup absorb chain")
                    dma_absorber = mmz
                ma_t = mabc_all[ich][:, j * BS:(j + 1) * BS]

                # Two PSUM banks per step, split by reader engine so the
                # bank-recycling matmul waits on at most {1 reader engine,
                # PE} (walrus allows only 2 sync waits per matmul):
                #   bank A = [zu|zr]  (read by ACT sigmoid only)
                #   bank B = [xc|sc]  (read by DVE only)
                # Openers read x (not s) so they carry no DVE wait; each
                # bank is one accumulation group (opener start=True zeroes
                # the bank lazily; the rest accumulate).
                pa = apsum.tile([D, 256], f32, name="pa", tag="pa")
                pbk = bpsum.tile([D, 256], f32, name="pbk", tag="pbk")
                ma1 = nc.tensor.matmul(pa[:, 0:128], lhsT=wt["wau"], rhs=x_t, start=True, stop=False)
                if j == 0:
                    # ensure the DMA-absorbing dummy runs before the openers
                    add_dep_helper(ma1.ins, dma_absorber.ins, sync=False, reason="chunk dma absorbed first")
                ma2 = nc.tensor.matmul(pa[:, 128:256], lhsT=wt["war"], rhs=x_t, start=False, stop=False)
                ma3 = nc.tensor.matmul(pa[:, 0:128], lhsT=wt["wbu"], rhs=s, start=False, stop=False)
                ma4 = nc.tensor.matmul(pa[:, 128:256], lhsT=wt["wbr"], rhs=s, start=False, stop=True)
                for a, b in zip([ma2, ma3, ma4], [ma1, ma2, ma3]):
                    add_dep_helper(a.ins, b.ins, sync=False, reason="bank A group order")
                mb1 = nc.tensor.matmul(pbk[:, 0:128], lhsT=wt["wac"], rhs=x_t, start=True, stop=False)
                if j == 0:
                    add_dep_helper(mb1.ins, dma_absorber.ins, sync=False, reason="chunk dma absorbed first")
                mb2 = nc.tensor.matmul(pbk[:, 128:256], lhsT=wt["wbc"], rhs=s, start=False, stop=True)
                add_dep_helper(mb2.ins, mb1.ins, sync=False, reason="bank B group order")

                ur = ew.tile([D, 256], f32, name="ur", tag="ur")
                if use_bias:
                    nc.scalar.activation(ur[:, 0:128], pa[:, 0:128], AF.Sigmoid, bias=bt["bau"])
                    nc.scalar.activation(ur[:, 128:256], pa[:, 128:256], AF.Sigmoid, bias=bt["bar"])
                else:
                    nc.scalar.activation(ur, pa[:, 0:256], AF.Sigmoid)

                rc = ew.tile([D, BS], f32, name="rc", tag="rc")
                nc.vector.tensor_mul(rc, ur[:, 128:256], pbk[:, 128:256])
                t2 = ew.tile([D, BS], f32, name="t2", tag="t2")
                nc.vector.tensor_add(t2, rc, pbk[:, 0:128])
                c = ew.tile([D, BS], f32, name="c", tag="c")
                if use_bias:
                    nc.scalar.activation(c, t2, AF.Tanh, bias=bt["bac"])
                else:
                    nc.scalar.activation(c, t2, AF.Tanh)

                dd = ew.tile([D, BS], f32, name="dd", tag="dd")
                nc.vector.tensor_sub(dd, c, s)
                ww = ew.tile([D, BS], f32, name="ww", tag="ww")
                nc.vector.tensor_mul(ww, ur[:, 0:128], dd)
                ee = ew.tile([D, BS], f32, name="ee", tag="ee")
                nc.vector.tensor_mul(ee, ww, ma_t)
                s_new = spool.tile([D, BS], f32, name="s", tag="s")
                nc.vector.tensor_add(s_new, s, ee)
                s = s_new

        nc.sync.dma_start(out=out_d[:, :], in_=s)

    nc.finalize()
    return nc


def _max_matmul_waits(nc):
    # walrus ISA structs have tight sync-wait budgets: a matmul (folded
    # into the LDWEIGHTS struct) holds ONE cross-engine wait (same-engine
    # PE waits are elided); other compute structs hold two waits total.
    worst = 0
    compute = ("InstMatmult", "InstLdweights", "InstTensorTensor",
               "InstTensorScalarPtr", "InstActivation", "InstMemset")
    for b in nc.m.functions[0].blocks:
        for ins in b.instructions:
            tn = type(ins).__name__
            if tn not in compute:
                continue
            si = ins.sync_info
            waits = list(si.on_wait) if si is not None else []
            if tn in ("InstMatmult", "InstLdweights"):
                n = sum(1 for w in waits if not str(w.ant_name).startswith("PE"))
                worst = max(worst, 2 if n > 1 else n)
            else:
                worst = max(worst, len(waits) - 1)
    return worst


def _get_program(use_bias: bool):
    key = use_bias
    if key not in _PROGRAM_CACHE:
        # The Tile scheduler is not deterministic across builds; walrus
        # rejects matmuls with >2 sync waits. Rebuild until the schedule
        # satisfies the limit.
        last = None
        for _ in range(12):
            nc = _build_program(use_bias)
            last = _max_matmul_waits(nc)
            if last <= 1:
                _PROGRAM_CACHE[key] = nc
                break
        else:
            raise RuntimeError(f"could not build a <=1-cross-wait schedule (last worst={last})")
    return _PROGRAM_CACHE[key]


def kernel(**inputs) -> np.ndarray:
    import os
    os.environ["BASS_NEVER_TRACE"] = "1"  # axon ntff hook unavailable here
    from concourse.bass_utils import run_bass_kernel_spmd

    x = np.asarray(inputs["inputs"], dtype=np.float32)      # [T, B, D]
    state = np.asarray(inputs["state"], dtype=np.float32)   # [B, D]
    att = np.asarray(inputs["att_score"], dtype=np.float32) # [T, B, 1]
    mask = np.asarray(inputs["mask"], dtype=np.float32)     # [B, T]
    weights = {n: np.ascontiguousarray(np.asarray(inputs[k], dtype=np.float32))
               for n, k in [("wau", "Wau"), ("war", "War"), ("wac", "Wac"),
                            ("wbu", "Wbu"), ("wbr", "Wbr"), ("wbc", "Wbc")]}
    biases = {n: np.asarray(inputs[k], dtype=np.float32)
              for n, k in [("bau", "bau"), ("bar", "bar"), ("bac", "bac")]}
    use_bias = any(np.any(b != 0.0) for b in biases.values())

    # ma[t, b] = att[t, b] * mask[b, t]
    ma = att[:, :, 0] * mask.T                               # [T, B]

    # x rearranged per core: [NCH, D, CH*BS]; batch is core-major blocks.
    # x[t, b, d] with t = ich*CH + j, b = c*BS + k  ->  xc[c, ich, d, j*BS + k]
    xr = x.reshape(NCH, CH, NCORES, BS, D).transpose(2, 0, 4, 1, 3)
    xr = np.ascontiguousarray(xr.reshape(NCORES, NCH, D, CH * BS), dtype=np.float32)

    sT = np.ascontiguousarray(state.T)                       # [D, B]

    nc = _get_program(use_bias)

    in_maps = []
    for c in range(NCORES):
        m = {
            "x": xr[c],
            "s0": np.ascontiguousarray(sT[:, c * BS:(c + 1) * BS]),
            "ma": np.ascontiguousarray(np.concatenate(
                [ma[:, c * BS:(c + 1) * BS].reshape(1, T * BS),
                 np.ones((1, D), np.float32)], axis=1)),
        }
        m["zconst"] = np.zeros((D, D), np.float32)
        m.update(weights)
        if use_bias:
            m.update({n: np.ascontiguousarray(b.reshape(D, 1)) for n, b in biases.items()})
        in_maps.append(m)

    res = run_bass_kernel_spmd(nc, in_maps, list(range(NCORES)))
    outs = [res.results[c]["sout"] for c in range(NCORES)]   # each [D, BS]
    full = np.concatenate(outs, axis=1)                      # [D, B]
    return np.ascontiguousarray(full.T).astype(np.float32)   # [B, D]
